# revision 29
# baseline (speedup 1.0000x reference)
"""BertEmbeddings (7-way embedding sum + Time2Vec + LayerNorm) on 8 TRN2 cores.

Redesign v3: all Time2Vec / sinusoidal-position math is computed on-chip
with the heavy lifting moved off the Vector engine:

  - The affine angle x = tau*w' + b' (in TURNS) is computed on the Tensor
    engine as a K=20 bf16 split-precision matmul (each operand split into
    2-3 bf16 parts whose pairwise products are exact) -> psT (PSUM f32).
  - Range reduction: ONE DVE tensor_scalar pass (x + 2^23) - 2^23 = round(x)
    (f32 RNE mantissa trick) -> k (fp16, exact for |k|<2048); then a second
    matmul with lhsT = -I accumulates -k INTO psT, leaving d = x - round(x)
    in [-0.5, 0.5].  No mod/cast/subtract passes on DVE.
  - ACT Sin reads PSUM directly: sin(2*pi*d), |arg| <= pi (LUT-valid).
    Position features with small angles (dims >= 271) skip reduction
    entirely (phases arranged so args always land in [-pi, pi]).
  - Word/NPI rows are dma_gather'ed as fp8(e4m3); sin outputs are fp8.
    The 7-way sum runs on TensorE: one K=64 one-hot matmul (modal+seg),
    one DoubleRow fp8 matmul summing word+npi in a single pass, one
    DoubleRow matmul summing age+del sins, one plain fp8 matmul for posi.
  - LayerNorm: bn_stats/bn_aggr (DVE), rstd via Quake-style bit-trick +
    2 Newton steps (no Sqrt ACT table -> the Sin table set stays loaded),
    normalize on ACT (scale/bias per partition), DMA out per column.

Self-contained: hardcodes shapes; kernel(**inputs) takes full unsharded
inputs, returns the full [8, 2048, 768] float32 output.
"""

import math

import numpy as np

B, S, H = 8, 2048, 768
VOCAB, MODAL_V, SEG_V, NPI_V, MAX_POS = 32000, 16, 4, 10000, 2048
P = 128
COLS = S // P            # 16 token-columns of 128 tokens
LN_EPS = 1e-12
TWO_PI = 2.0 * math.pi
KMS = 64                 # modal rows 0..15, seg rows 32..35, pad
K20 = 20                 # t2v matmul contraction rows
HA = H - 1               # 767 sin dims per t2v table
JLO = 271                # posi dims < JLO need range reduction
NLO = JLO                # width of posi_lo block
TW = HA + HA + H         # 2302 total t2v/posi slots: [age|del|posi_lo|posi_hi]
MODW = HA + HA + NLO     # 1805 slots that need range reduction
ROUNDS = [(0, 1024), (1024, 2048), (2048, TW)]
MAGIC = 0x5F3759DF
DEBUG_DUMP = False
LDW_OPT = False  # walrus ldw-opt crashes codegen (visitInstLdweights); keep off

_cache = {}


def _enable_ldw_opt():
    import concourse.bass_utils as bu

    if getattr(bu, "_ldw_patched", False):
        return
    orig = bu.run_command

    def run_command_ldw(cmd, *a, **kw):
        if isinstance(cmd, list):
            cmd = [("--enable-ldw-opt=true" if c == "--enable-ldw-opt=false" else c)
                   for c in cmd]
        return orig(cmd, *a, **kw)

    bu.run_command = run_command_ldw
    bu._ldw_patched = True


def _resplit_last(ap_obj, groups, width):
    """[P, N] AP -> [P, groups, width] AP (N = groups*width, contiguous)."""
    import concourse.bass as bass

    a = ap_obj
    assert a.ap[-1][0] == 1
    return bass.AP(
        tensor=a.tensor, offset=a.offset,
        ap=[a.ap[0], [width, groups], [1, width]],
    )


def _build(use_gamma_beta: bool):
    if LDW_OPT:
        _enable_ldw_opt()
    import concourse.bacc as bacc
    import concourse.bass as bass
    import concourse.tile as tile
    from concourse import mybir
    from contextlib import ExitStack

    f32 = mybir.dt.float32
    bf16 = mybir.dt.bfloat16
    fp16 = mybir.dt.float16
    fp8 = mybir.dt.float8e4
    i32 = mybir.dt.int32
    i16 = mybir.dt.int16
    Alu = mybir.AluOpType
    Act = mybir.ActivationFunctionType
    DR = mybir.MatmulPerfMode.DoubleRow

    nc = bacc.Bacc("TRN2", target_bir_lowering=False, debug=False,
                   dynamic_dma_scratch_size=24576, num_swdge_queues=2)

    d_lhsT = nc.dram_tensor("lhsT20", [K20, S], bf16, kind="ExternalInput")
    d_rhs = nc.dram_tensor("rhs20", [K20, TW], bf16, kind="ExternalInput")
    d_widx = nc.dram_tensor("word_idx16", [P, 4, S // 4 // 16], i16, kind="ExternalInput")
    d_nidx = nc.dram_tensor("npi_idx16", [P, 4, S // 4 // 16], i16, kind="ExternalInput")
    d_word = nc.dram_tensor("word_table", [VOCAB, H], fp8, kind="ExternalInput")
    d_npi = nc.dram_tensor("npi_table", [NPI_V, H], fp8, kind="ExternalInput")
    d_ctbl = nc.dram_tensor("ctbl", [KMS, H], bf16, kind="ExternalInput")
    d_modal_ids = nc.dram_tensor("modalities_ids", [S], i32, kind="ExternalInput")
    d_seg_ids = nc.dram_tensor("seg_ids", [S], i32, kind="ExternalInput")
    d_iota = nc.dram_tensor("iota64", [KMS, 1], i32, kind="ExternalInput")
    d_v2t = nc.dram_tensor("v2t", [P, COLS], f32, kind="ExternalInput")
    d_id2 = nc.dram_tensor("ident2", [P, 2, P], fp8, kind="ExternalInput")
    d_idp = nc.dram_tensor("identp", [P, P], fp8, kind="ExternalInput")
    d_negi = nc.dram_tensor("negident", [P, P], fp16, kind="ExternalInput")
    if use_gamma_beta:
        d_gamma = nc.dram_tensor("ln_gamma", [H], f32, kind="ExternalInput")
        d_beta = nc.dram_tensor("ln_beta", [H], f32, kind="ExternalInput")
    d_out = nc.dram_tensor("out", [S, H], f32, kind="ExternalOutput")
    if DEBUG_DUMP:
        d_dbg_sin = nc.dram_tensor("dbg_sin", [P, TW], f32, kind="ExternalOutput")
        d_dbg_emb = nc.dram_tensor("dbg_emb", [P, H], f32, kind="ExternalOutput")
        d_dbg_mv = nc.dram_tensor("dbg_mv", [P, 2, 2], f32, kind="ExternalOutput")
        d_dbg_wn = nc.dram_tensor("dbg_wn", [P, 2, H], f32, kind="ExternalOutput")
        d_dbg_oh = nc.dram_tensor("dbg_oh", [KMS, P], f32, kind="ExternalOutput")
        d_dbg_k = nc.dram_tensor("dbg_k", [P, 1024], f32, kind="ExternalOutput")
        d_dbg_d = nc.dram_tensor("dbg_d", [P, 1024], f32, kind="ExternalOutput")

    def bcast_rows(handle, n, count, offset=0):
        ap = handle.ap()
        return bass.AP(tensor=ap.tensor, offset=offset, ap=[[0, n], [1, count]])

    with tile.TileContext(nc) as tc, ExitStack() as ctx:
        singles = ctx.enter_context(tc.tile_pool(name="singles", bufs=1))
        sins = ctx.enter_context(tc.tile_pool(name="sins", bufs=4))
        kpool = ctx.enter_context(tc.tile_pool(name="kpool", bufs=3))
        opool = ctx.enter_context(tc.tile_pool(name="opool", bufs=6))
        small = ctx.enter_context(tc.tile_pool(name="small", bufs=2))
        pst = ctx.enter_context(tc.tile_pool(name="pst", bufs=3, space="PSUM"))

        # ---- static tiles ----
        lhsT = singles.tile([K20, COLS, P], bf16)
        nc.sync.dma_start(out=lhsT[:], in_=d_lhsT.ap().rearrange("k (c p) -> k c p", p=P))
        rhsT = singles.tile([K20, TW], bf16)
        nc.sync.dma_start(out=rhsT[:], in_=d_rhs.ap())
        id2 = singles.tile([P, 2, P], fp8)
        nc.sync.dma_start(out=id2[:], in_=d_id2.ap())
        idp = singles.tile([P, P], fp8)
        nc.sync.dma_start(out=idp[:], in_=d_idp.ap())
        negi = singles.tile([P, P], fp16)
        nc.sync.dma_start(out=negi[:], in_=d_negi.ap())
        ctbl = singles.tile([KMS, H], bf16)
        nc.sync.dma_start(out=ctbl[:], in_=d_ctbl.ap())
        v2t = singles.tile([P, COLS], f32)
        nc.sync.dma_start(out=v2t[:], in_=d_v2t.ap())
        iota = singles.tile([KMS, 1], i32)
        nc.sync.dma_start(out=iota[:], in_=d_iota.ap())
        magic = singles.tile([P, 2], i32)
        nc.vector.memset(magic[:], MAGIC)
        if use_gamma_beta:
            gamma_t = singles.tile([P, H], f32)
            beta_t = singles.tile([P, H], f32)
            nc.sync.dma_start(out=gamma_t[:], in_=bcast_rows(d_gamma, P, H))
            nc.sync.dma_start(out=beta_t[:], in_=bcast_rows(d_beta, P, H))

        # gather indices
        wi16 = singles.tile([P, 4, S // 4 // 16], i16)
        ni16 = singles.tile([P, 4, S // 4 // 16], i16)
        nc.sync.dma_start(out=wi16[:], in_=d_widx.ap())
        nc.sync.dma_start(out=ni16[:], in_=d_nidx.ap())

        # modal/seg ids broadcast to the one-hot orientation, one-hot built once
        ids_all = singles.tile([KMS, COLS, P], i32)
        nc.sync.dma_start(
            out=ids_all[0:MODAL_V, :, :],
            in_=bass.AP(tensor=d_modal_ids.ap().tensor, offset=0,
                        ap=[[0, MODAL_V], [P, COLS], [1, P]]),
        )
        nc.sync.dma_start(
            out=ids_all[32:32 + SEG_V, :, :],
            in_=bass.AP(tensor=d_seg_ids.ap().tensor, offset=0,
                        ap=[[0, SEG_V], [P, COLS], [1, P]]),
        )
        # rows not in {0..15, 32..35} compare against -1 -> all zeros
        nc.sync.dma_start(
            out=ids_all[16:32, :, :],
            in_=bass.AP(tensor=d_modal_ids.ap().tensor, offset=0,
                        ap=[[0, 16], [P, COLS], [1, P]]),
        )
        nc.sync.dma_start(
            out=ids_all[36:KMS, :, :],
            in_=bass.AP(tensor=d_modal_ids.ap().tensor, offset=0,
                        ap=[[0, KMS - 36], [P, COLS], [1, P]]),
        )
        onehot = singles.tile([KMS, COLS, P], bf16)
        iota_b = bass.AP(tensor=iota[:].tensor, offset=iota[:].offset,
                         ap=[iota[:].ap[0], [0, COLS], [0, P]])
        nc.vector.tensor_tensor(out=onehot[:], in0=ids_all[:], in1=iota_b, op=Alu.is_equal)

        # word/npi gathered rows, fp8, [P, {word,npi}, COLS, H]
        wn = singles.tile([P, 2, COLS, H], fp8)
        NG = S // 4  # 512 idxs per gather chunk
        for quarter in range(4):
            for gi, (tbl, idxs) in enumerate(((d_word, wi16), (d_npi, ni16))):
                nc.gpsimd.dma_gather(
                    out_ap=wn[:, gi, quarter * 4:(quarter + 1) * 4, :],
                    in_ap=tbl.ap(), idxs_ap=idxs[:, quarter, :],
                    num_idxs=NG, num_idxs_reg=NG, elem_size=H,
                    queue_num=gi,
                )

        def affine_segs(r0, r1):
            w = r1 - r0
            mod_end = max(r0, min(r1, MODW)) - r0
            cuts = sorted({0, w} | {n for n in (512, 1024) if 0 < n < w} | ({mod_end} if 0 < mod_end < w else set()))
            return list(zip(cuts[:-1], cuts[1:])), mod_end

        # Software-pipelined main loop: for each column, the Time2Vec
        # affine/reduce/sin runs one column AHEAD of the embedding-sum +
        # LayerNorm, so the PE always has ready matmul work while DVE/ACT
        # process the previous stage.  All PSUM tiles rotate through one
        # 4-buffer tag (4 x 2 banks = all 8 PSUM banks).
        NCHUNKS = ((0, 512), (512, H))
        sin_of, ps_of = {}, {}

        def t2v_front(c):
            """affines (R3,R1,R2) + sin-R3 + k-passes for column c."""
            sinS = sins.tile([P, TW], fp8, tag="sinS")
            ps_r, kt_r, me_r, segs_r = {}, {}, {}, {}
            for ri in (2, 0, 1):
                r0, r1 = ROUNDS[ri]
                ps = pst.tile([P, 1024], f32, tag="ps")
                ps_r[ri] = ps
                segs, mod_end = affine_segs(r0, r1)
                me_r[ri], segs_r[ri] = mod_end, segs
                last_in_reg = {}
                for (a0, a1) in segs:
                    last_in_reg[a0 // 512] = a0
                seen = set()
                for (a0, a1) in segs:
                    reg = a0 // 512
                    nc.tensor.matmul(
                        out=ps[:, a0:a1], lhsT=lhsT[:, c, :],
                        rhs=rhsT[:, r0 + a0:r0 + a1],
                        start=reg not in seen, stop=last_in_reg[reg] == a0,
                    )
                    seen.add(reg)
                if ri == 2:
                    nc.scalar.activation(
                        out=sinS[:, r0:r1], in_=ps[:, 0:r1 - r0],
                        func=Act.Sin, scale=TWO_PI,
                    )
                else:
                    # k-pass immediately after each round's affine so the
                    # DVE starts while the PE streams the next round
                    kt = kpool.tile([P, 1024], fp16, tag="kS")
                    kt_r[ri] = kt
                    nc.vector.tensor_scalar(
                        out=kt[:, 0:mod_end], in0=ps[:, 0:mod_end],
                        scalar1=12582912.0, scalar2=12582912.0,
                        op0=Alu.add, op1=Alu.subtract,
                    )
            return sinS, (ps_r, kt_r, me_r, segs_r)

        def t2v_back(c):
            """-I subtract matmuls + sins for column c."""
            ps_r, kt_r, me_r, segs_r = ps_of[c]
            for ri in (0, 1):
                for (a0, a1) in segs_r[ri]:
                    if a0 >= me_r[ri]:
                        break
                    nc.tensor.matmul(
                        out=ps_r[ri][:, a0:a1], lhsT=negi[:],
                        rhs=kt_r[ri][:, a0:a1], start=False, stop=True,
                        skip_group_check=True,
                    )
            for ri in (0, 1):
                r0, r1 = ROUNDS[ri]
                nc.scalar.activation(
                    out=sin_of[c][:, r0:r1], in_=ps_r[ri][:, 0:r1 - r0],
                    func=Act.Sin, scale=TWO_PI,
                )

        def emb_ln(c):
            """7-way sum on TensorE + LayerNorm + store for column c."""
            sinS = sin_of.pop(c)
            psE = pst.tile([P, 1024], f32, tag="psE", bufs=1)
            for n0, n1 in NCHUNKS:
                nc.tensor.matmul(
                    out=psE[:, n0:n1], lhsT=onehot[:, c, :],
                    rhs=ctbl[:, n0:n1], start=True, stop=False,
                )
            wn_ap = wn[:]
            s_ap = sinS[:]
            for n0, n1 in NCHUNKS:
                n = n1 - n0
                nc.tensor.matmul(
                    out=psE[:, n0:n1],
                    lhsT=id2[:],
                    rhs=bass.AP(tensor=wn_ap.tensor,
                                offset=wn_ap.offset + c * H + n0,
                                ap=[wn_ap.ap[0], [COLS * H, 2], [1, n]]),
                    start=False, stop=False, perf_mode=DR,
                )
                na = min(n1, HA) - n0  # age/del sins are 767 wide
                nc.tensor.matmul(
                    out=psE[:, n0:n0 + na],
                    lhsT=id2[:],
                    rhs=bass.AP(tensor=s_ap.tensor, offset=s_ap.offset + n0,
                                ap=[s_ap.ap[0], [HA, 2], [1, na]]),
                    start=False, stop=False, perf_mode=DR,
                )
            for n0, n1 in NCHUNKS:
                nc.tensor.matmul(
                    out=psE[:, n0:n1], lhsT=idp[:],
                    rhs=sinS[:, 2 * HA + n0:2 * HA + n1],
                    start=False, stop=True,
                )
            nc.vector.tensor_tensor(
                out=psE[:, H - 1:H], in0=psE[:, H - 1:H],
                in1=v2t[:, c:c + 1], op=Alu.add,
            )
            # LN stats + rsqrt(var) via bit-trick + 1 Newton step
            st = small.tile([P, 2, 6], f32, tag="bnst")
            nc.vector.bn_stats(out=st[:, 0, :], in_=psE[:, 0:H // 2])
            nc.vector.bn_stats(out=st[:, 1, :], in_=psE[:, H // 2:H])
            mv = small.tile([P, 2], f32, tag="mv")
            nc.vector.bn_aggr(out=mv[:], in_=st[:].rearrange("p a b -> p (a b)"))
            vv = mv[:, 1:2]
            y = small.tile([P, 1], f32, tag="rsq_y")
            t = small.tile([P, 1], f32, tag="rsq_t")
            nc.vector.tensor_scalar(
                out=y[:].bitcast(i32), in0=vv.bitcast(i32),
                scalar1=1, scalar2=None, op0=Alu.logical_shift_right,
            )
            nc.vector.tensor_tensor(
                out=y[:].bitcast(i32), in0=magic[:, 0:1], in1=y[:].bitcast(i32),
                op=Alu.subtract,
            )
            # two fused Newton steps: t = (v*y)*y ; t = 1.5 - 0.5*t ; y *= t
            for _ in range(2):
                nc.vector.scalar_tensor_tensor(
                    out=t[:], in0=vv, scalar=y[:], in1=y[:],
                    op0=Alu.mult, op1=Alu.mult,
                )
                nc.vector.tensor_scalar(
                    out=t[:], in0=t[:], scalar1=-0.5, scalar2=1.5,
                    op0=Alu.mult, op1=Alu.add,
                )
                nc.vector.tensor_tensor(out=y[:], in0=y[:], in1=t[:], op=Alu.mult)
            nmu = small.tile([P, 1], f32, tag="nmu")
            nc.vector.scalar_tensor_tensor(
                out=nmu[:], in0=mv[:, 0:1], scalar=-1.0, in1=y[:],
                op0=Alu.mult, op1=Alu.mult,
            )
            outS = opool.tile([P, H], f32, tag="outS")
            nc.scalar.activation(
                out=outS[:], in_=psE[:, 0:H], func=Act.Identity,
                scale=y[:], bias=nmu[:],
            )
            if use_gamma_beta:
                nc.vector.tensor_tensor(out=outS[:], in0=outS[:], in1=gamma_t[:], op=Alu.mult)
                nc.vector.tensor_tensor(out=outS[:], in0=outS[:], in1=beta_t[:], op=Alu.add)
            nc.sync.dma_start(
                out=d_out.ap().rearrange("(c p) h -> p c h", p=P)[:, c, :],
                in_=outS[:],
            )

        for c in range(COLS + 1):
            if c < COLS:
                sin_of[c], ps_of[c] = t2v_front(c)
            if c >= 1:
                emb_ln(c - 1)
            if c < COLS:
                t2v_back(c)
                ps_of.pop(c, None)

    nc.compile()
    return nc


def _get_nc(use_gamma_beta: bool):
    key = ("nc", use_gamma_beta)
    if key not in _cache:
        _cache[key] = _build(use_gamma_beta)
    return _cache[key]


def _f32a(x):
    return np.ascontiguousarray(np.asarray(x), dtype=np.float32)


def _i32a(x):
    return np.ascontiguousarray(np.asarray(x), dtype=np.int32)


def _pack_idx16(ids_row):
    # [S] -> [P, 4, 32]: idx position i of quarter q at [i % 16, q, i // 16], x8.
    QN = S // 4
    arr = np.zeros((16, 4, QN // 16), dtype=np.int16)
    for q in range(4):
        blk = ids_row[q * QN:(q + 1) * QN].reshape(QN // 16, 16)
        arr[:, q, :] = blk.T.astype(np.int16)
    return np.ascontiguousarray(np.tile(arr, (8, 1, 1)))


def _split3(x, bf):
    x = np.asarray(x, dtype=np.float64)
    p1 = x.astype(bf)
    r = x - p1.astype(np.float64)
    p2 = r.astype(bf)
    p3 = (r - p2.astype(np.float64)).astype(bf)
    return p1, p2, p3


_last_use_gb = False


def _make_in_maps(inputs, use_gb):
    import ml_dtypes
    bf = ml_dtypes.bfloat16
    f8 = ml_dtypes.float8_e4m3

    word_ids = _i32a(inputs["word_ids"]).reshape(B, S)
    modal_ids = _i32a(inputs["modalities_ids"]).reshape(B, S)
    seg_ids = _i32a(inputs["seg_ids"]).reshape(B, S)
    npi_ids = _i32a(inputs["npi_ids"]).reshape(B, S)
    posi_ids = _i32a(inputs["posi_ids"]).reshape(B, S)
    age_tau = _f32a(inputs["age_tau"]).reshape(B, S)
    delay_tau = _f32a(inputs["delays_tau"]).reshape(B, S)

    # ---- rhs20: split-precision weight rows (shared across cores) ----
    aw = _f32a(inputs["age_w"]).reshape(HA) / TWO_PI
    ab = _f32a(inputs["age_b"]).reshape(HA) / TWO_PI
    dw = _f32a(inputs["delay_w"]).reshape(HA) / TWO_PI
    db = _f32a(inputs["delay_b"]).reshape(HA) / TWO_PI
    j = np.arange(H, dtype=np.float64)
    omega = (10000.0 ** (-2.0 * j / H)) / TWO_PI      # turns per unit pos
    sign = np.where(j % 2 == 0, 1.0, -1.0)            # odd dims: cos via 0.25 - x
    phase = np.where(j % 2 == 0, 0.0, 0.25)
    som = sign * omega
    # slot order: lo dims (j < JLO) then hi dims
    order = np.concatenate([j[:JLO], j[JLO:]]).astype(np.int64)
    som_s, phase_s = som[order], phase[order]

    aw1, aw2, aw3 = _split3(aw, bf)
    dw1, dw2, dw3 = _split3(dw, bf)
    ab1, ab2, ab3 = _split3(ab, bf)
    db1, db2, db3 = _split3(db, bf)
    om1, om2, om3 = _split3(som_s, bf)

    rhs = np.zeros((K20, TW), dtype=bf)
    rhs[0, 0:HA], rhs[1, 0:HA], rhs[2, 0:HA] = aw1, aw2, aw3
    rhs[3, 0:HA], rhs[4, 0:HA] = aw1, aw2
    rhs[5, 0:HA] = aw1
    rhs[6, HA:2 * HA], rhs[7, HA:2 * HA], rhs[8, HA:2 * HA] = dw1, dw2, dw3
    rhs[9, HA:2 * HA], rhs[10, HA:2 * HA] = dw1, dw2
    rhs[11, HA:2 * HA] = dw1
    rhs[12, 2 * HA:], rhs[13, 2 * HA:], rhs[14, 2 * HA:] = om1, om2, om3
    rhs[15, 2 * HA:], rhs[16, 2 * HA:] = om1, om2
    rhs[17, 0:HA], rhs[18, 0:HA], rhs[19, 0:HA] = ab1, ab2, ab3
    rhs[17, HA:2 * HA], rhs[18, HA:2 * HA], rhs[19, HA:2 * HA] = db1, db2, db3
    rhs[17, 2 * HA:] = phase_s.astype(bf)

    # combined modal+seg table
    ctbl = np.zeros((KMS, H), dtype=bf)
    ctbl[0:MODAL_V] = _f32a(inputs["modalities_table"]).reshape(MODAL_V, H).astype(bf)
    ctbl[32:32 + SEG_V] = _f32a(inputs["seg_table"]).reshape(SEG_V, H).astype(bf)
    iota64 = np.full((KMS, 1), -1, dtype=np.int32)
    iota64[0:MODAL_V, 0] = np.arange(MODAL_V)
    iota64[32:32 + SEG_V, 0] = np.arange(SEG_V)

    id2 = np.zeros((P, 2, P), dtype=f8)
    eye = np.eye(P, dtype=np.float32)
    id2[:, 0, :] = eye.astype(f8)
    id2[:, 1, :] = eye.astype(f8)
    identp = np.ascontiguousarray(eye.astype(f8))
    negi = np.ascontiguousarray((-eye).astype(np.float16))

    shared = {
        "rhs20": np.ascontiguousarray(rhs),
        "word_table": np.ascontiguousarray(
            _f32a(inputs["word_table"]).reshape(VOCAB, H).astype(f8)),
        "npi_table": np.ascontiguousarray(
            _f32a(inputs["npi_table"]).reshape(NPI_V, H).astype(f8)),
        "ctbl": np.ascontiguousarray(ctbl),
        "iota64": iota64,
        "ident2": np.ascontiguousarray(id2),
        "identp": identp,
        "negident": negi,
    }
    if use_gb:
        shared["ln_gamma"] = _f32a(inputs["ln_gamma"]).reshape(H)
        shared["ln_beta"] = _f32a(inputs["ln_beta"]).reshape(H)

    aw0 = float(_f32a(inputs["age_w0"]).reshape(()))
    ab0 = float(_f32a(inputs["age_b0"]).reshape(()))
    dw0 = float(_f32a(inputs["delay_w0"]).reshape(()))
    db0 = float(_f32a(inputs["delay_b0"]).reshape(()))

    in_maps = []
    for i in range(B):
        ta = age_tau[i].astype(np.float64)
        td = delay_tau[i].astype(np.float64)
        pos = posi_ids[i].astype(np.float64)
        t1a, t2a, t3a = _split3(ta, bf)
        t1d, t2d, t3d = _split3(td, bf)
        p1 = pos.astype(bf)
        p2 = (pos - p1.astype(np.float64)).astype(bf)
        ones = np.ones(S, dtype=bf)
        lhsT = np.stack([
            t1a, t1a, t1a, t2a, t2a, t3a,
            t1d, t1d, t1d, t2d, t2d, t3d,
            p1, p1, p1, p2, p2,
            ones, ones, ones,
        ]).astype(bf)
        v2 = (ta * aw0 + ab0 + td * dw0 + db0).astype(np.float32)
        v2t = np.ascontiguousarray(v2.reshape(COLS, P).T)
        m = dict(shared)
        m.update(
            lhsT20=np.ascontiguousarray(lhsT),
            word_idx16=_pack_idx16(word_ids[i]),
            npi_idx16=_pack_idx16(npi_ids[i]),
            modalities_ids=modal_ids[i],
            seg_ids=seg_ids[i],
            v2t=v2t,
        )
        in_maps.append(m)
    return in_maps


def kernel(**inputs) -> np.ndarray:
    global _last_use_gb
    from concourse.bass_utils import run_bass_kernel_spmd

    gamma = _f32a(inputs["ln_gamma"])
    beta = _f32a(inputs["ln_beta"])
    use_gb = not (np.all(gamma == 1.0) and np.all(beta == 0.0))
    _last_use_gb = use_gb
    nc = _get_nc(use_gb)
    in_maps = _make_in_maps(inputs, use_gb)
    core_ids = list(range(B))
    res = run_bass_kernel_spmd(nc, in_maps, core_ids)
    out = np.stack([res.results[i]["out"] for i in core_ids], axis=0)
    return out


# revision 30
# speedup vs baseline: 1.3129x; 1.3129x over previous
"""BertEmbeddings (7-way embedding sum + Time2Vec + LayerNorm) on 8 TRN2 cores.

Redesign v3: all Time2Vec / sinusoidal-position math is computed on-chip
with the heavy lifting moved off the Vector engine:

  - The affine angle x = tau*w' + b' (in TURNS) is computed on the Tensor
    engine as a K=20 bf16 split-precision matmul (each operand split into
    2-3 bf16 parts whose pairwise products are exact) -> psT (PSUM f32).
  - Range reduction: ONE DVE tensor_scalar pass (x + 2^23) - 2^23 = round(x)
    (f32 RNE mantissa trick) -> k (fp16, exact for |k|<2048); then a second
    matmul with lhsT = -I accumulates -k INTO psT, leaving d = x - round(x)
    in [-0.5, 0.5].  No mod/cast/subtract passes on DVE.
  - ACT Sin reads PSUM directly: sin(2*pi*d), |arg| <= pi (LUT-valid).
    Position features with small angles (dims >= 271) skip reduction
    entirely (phases arranged so args always land in [-pi, pi]).
  - Word/NPI rows are dma_gather'ed as fp8(e4m3); sin outputs are fp8.
    The 7-way sum runs on TensorE: one K=64 one-hot matmul (modal+seg),
    one DoubleRow fp8 matmul summing word+npi in a single pass, one
    DoubleRow matmul summing age+del sins, one plain fp8 matmul for posi.
  - LayerNorm: bn_stats/bn_aggr (DVE), rstd via Quake-style bit-trick +
    2 Newton steps (no Sqrt ACT table -> the Sin table set stays loaded),
    normalize on ACT (scale/bias per partition), DMA out per column.

Self-contained: hardcodes shapes; kernel(**inputs) takes full unsharded
inputs, returns the full [8, 2048, 768] float32 output.
"""

import math

import numpy as np

B, S, H = 8, 2048, 768
VOCAB, MODAL_V, SEG_V, NPI_V, MAX_POS = 32000, 16, 4, 10000, 2048
P = 128
COLS = S // P            # 16 token-columns of 128 tokens
LN_EPS = 1e-12
TWO_PI = 2.0 * math.pi
KMS = 64                 # modal rows 0..15, seg rows 32..35, pad
K20 = 20                 # t2v matmul contraction rows
HA = H - 1               # 767 sin dims per t2v table
JLO = 271                # posi dims < JLO need range reduction
NLO = JLO                # width of posi_lo block
TW = HA + HA + H         # 2302 total t2v/posi slots: [age|del|posi_lo|posi_hi]
MODW = HA + HA + NLO     # 1805 slots that need range reduction
ROUNDS = [(0, 1024), (1024, 2048), (2048, TW)]
MAGIC = 0x5F3759DF
DEBUG_DUMP = False
LDW_OPT = False  # walrus ldw-opt crashes codegen (visitInstLdweights); keep off

_cache = {}


def _enable_ldw_opt():
    import concourse.bass_utils as bu

    if getattr(bu, "_ldw_patched", False):
        return
    orig = bu.run_command

    def run_command_ldw(cmd, *a, **kw):
        if isinstance(cmd, list):
            cmd = [("--enable-ldw-opt=true" if c == "--enable-ldw-opt=false" else c)
                   for c in cmd]
        return orig(cmd, *a, **kw)

    bu.run_command = run_command_ldw
    bu._ldw_patched = True


def _resplit_last(ap_obj, groups, width):
    """[P, N] AP -> [P, groups, width] AP (N = groups*width, contiguous)."""
    import concourse.bass as bass

    a = ap_obj
    assert a.ap[-1][0] == 1
    return bass.AP(
        tensor=a.tensor, offset=a.offset,
        ap=[a.ap[0], [width, groups], [1, width]],
    )


def _build(use_gamma_beta: bool):
    if LDW_OPT:
        _enable_ldw_opt()
    import concourse.bacc as bacc
    import concourse.bass as bass
    import concourse.tile as tile
    from concourse import mybir
    from contextlib import ExitStack

    f32 = mybir.dt.float32
    bf16 = mybir.dt.bfloat16
    fp16 = mybir.dt.float16
    fp8 = mybir.dt.float8e4
    i32 = mybir.dt.int32
    i16 = mybir.dt.int16
    Alu = mybir.AluOpType
    Act = mybir.ActivationFunctionType
    DR = mybir.MatmulPerfMode.DoubleRow

    nc = bacc.Bacc("TRN2", target_bir_lowering=False, debug=False,
                   dynamic_dma_scratch_size=24576, num_swdge_queues=2)

    d_lhsT = nc.dram_tensor("lhsT20", [K20, S], bf16, kind="ExternalInput")
    d_rhs = nc.dram_tensor("rhs20", [K20, TW], bf16, kind="ExternalInput")
    d_widx = nc.dram_tensor("word_idx16", [P, 4, S // 4 // 16], i16, kind="ExternalInput")
    d_nidx = nc.dram_tensor("npi_idx16", [P, 4, S // 4 // 16], i16, kind="ExternalInput")
    d_word = nc.dram_tensor("word_table", [VOCAB, H], fp8, kind="ExternalInput")
    d_npi = nc.dram_tensor("npi_table", [NPI_V, H], fp8, kind="ExternalInput")
    d_ctbl = nc.dram_tensor("ctbl", [KMS, H], bf16, kind="ExternalInput")
    d_modal_ids = nc.dram_tensor("modalities_ids", [S], i32, kind="ExternalInput")
    d_seg_ids = nc.dram_tensor("seg_ids", [S], i32, kind="ExternalInput")
    d_iota = nc.dram_tensor("iota64", [KMS, 1], i32, kind="ExternalInput")
    d_v2t = nc.dram_tensor("v2t", [P, COLS], f32, kind="ExternalInput")
    d_id2 = nc.dram_tensor("ident2", [P, 2, P], fp8, kind="ExternalInput")
    d_idp = nc.dram_tensor("identp", [P, P], fp8, kind="ExternalInput")
    d_negi = nc.dram_tensor("negident", [P, P], fp16, kind="ExternalInput")
    if use_gamma_beta:
        d_gamma = nc.dram_tensor("ln_gamma", [H], f32, kind="ExternalInput")
        d_beta = nc.dram_tensor("ln_beta", [H], f32, kind="ExternalInput")
    d_out = nc.dram_tensor("out", [S, H], f32, kind="ExternalOutput")
    if DEBUG_DUMP:
        d_dbg_sin = nc.dram_tensor("dbg_sin", [P, TW], f32, kind="ExternalOutput")
        d_dbg_emb = nc.dram_tensor("dbg_emb", [P, H], f32, kind="ExternalOutput")
        d_dbg_mv = nc.dram_tensor("dbg_mv", [P, 2, 2], f32, kind="ExternalOutput")
        d_dbg_wn = nc.dram_tensor("dbg_wn", [P, 2, H], f32, kind="ExternalOutput")
        d_dbg_oh = nc.dram_tensor("dbg_oh", [KMS, P], f32, kind="ExternalOutput")
        d_dbg_k = nc.dram_tensor("dbg_k", [P, 1024], f32, kind="ExternalOutput")
        d_dbg_d = nc.dram_tensor("dbg_d", [P, 1024], f32, kind="ExternalOutput")

    def bcast_rows(handle, n, count, offset=0):
        ap = handle.ap()
        return bass.AP(tensor=ap.tensor, offset=offset, ap=[[0, n], [1, count]])

    with tile.TileContext(nc) as tc, ExitStack() as ctx:
        singles = ctx.enter_context(tc.tile_pool(name="singles", bufs=1))
        sins = ctx.enter_context(tc.tile_pool(name="sins", bufs=4))
        kpool = ctx.enter_context(tc.tile_pool(name="kpool", bufs=3))
        opool = ctx.enter_context(tc.tile_pool(name="opool", bufs=6))
        small = ctx.enter_context(tc.tile_pool(name="small", bufs=2))
        pst = ctx.enter_context(tc.tile_pool(name="pst", bufs=4, space="PSUM"))

        # ---- static tiles ----
        lhsT = singles.tile([K20, COLS, P], bf16)
        nc.sync.dma_start(out=lhsT[:], in_=d_lhsT.ap().rearrange("k (c p) -> k c p", p=P))
        rhsT = singles.tile([K20, TW], bf16)
        nc.sync.dma_start(out=rhsT[:], in_=d_rhs.ap())
        id2 = singles.tile([P, 2, P], fp8)
        nc.sync.dma_start(out=id2[:], in_=d_id2.ap())
        idp = singles.tile([P, P], fp8)
        nc.sync.dma_start(out=idp[:], in_=d_idp.ap())
        negi = singles.tile([P, P], fp16)
        nc.sync.dma_start(out=negi[:], in_=d_negi.ap())
        ctbl = singles.tile([KMS, H], bf16)
        nc.sync.dma_start(out=ctbl[:], in_=d_ctbl.ap())
        v2t = singles.tile([P, COLS], f32)
        nc.sync.dma_start(out=v2t[:], in_=d_v2t.ap())
        iota = singles.tile([KMS, 1], i32)
        nc.sync.dma_start(out=iota[:], in_=d_iota.ap())
        magic = singles.tile([P, 2], i32)
        nc.vector.memset(magic[:], MAGIC)
        if use_gamma_beta:
            gamma_t = singles.tile([P, H], f32)
            beta_t = singles.tile([P, H], f32)
            nc.sync.dma_start(out=gamma_t[:], in_=bcast_rows(d_gamma, P, H))
            nc.sync.dma_start(out=beta_t[:], in_=bcast_rows(d_beta, P, H))

        # gather indices
        wi16 = singles.tile([P, 4, S // 4 // 16], i16)
        ni16 = singles.tile([P, 4, S // 4 // 16], i16)
        nc.sync.dma_start(out=wi16[:], in_=d_widx.ap())
        nc.sync.dma_start(out=ni16[:], in_=d_nidx.ap())

        # modal/seg ids broadcast to the one-hot orientation, one-hot built once
        ids_all = singles.tile([KMS, COLS, P], i32)
        nc.sync.dma_start(
            out=ids_all[0:MODAL_V, :, :],
            in_=bass.AP(tensor=d_modal_ids.ap().tensor, offset=0,
                        ap=[[0, MODAL_V], [P, COLS], [1, P]]),
        )
        nc.sync.dma_start(
            out=ids_all[32:32 + SEG_V, :, :],
            in_=bass.AP(tensor=d_seg_ids.ap().tensor, offset=0,
                        ap=[[0, SEG_V], [P, COLS], [1, P]]),
        )
        # rows not in {0..15, 32..35} compare against -1 -> all zeros
        nc.sync.dma_start(
            out=ids_all[16:32, :, :],
            in_=bass.AP(tensor=d_modal_ids.ap().tensor, offset=0,
                        ap=[[0, 16], [P, COLS], [1, P]]),
        )
        nc.sync.dma_start(
            out=ids_all[36:KMS, :, :],
            in_=bass.AP(tensor=d_modal_ids.ap().tensor, offset=0,
                        ap=[[0, KMS - 36], [P, COLS], [1, P]]),
        )
        onehot = singles.tile([KMS, COLS, P], bf16)
        iota_b = bass.AP(tensor=iota[:].tensor, offset=iota[:].offset,
                         ap=[iota[:].ap[0], [0, COLS], [0, P]])
        nc.vector.tensor_tensor(out=onehot[:], in0=ids_all[:], in1=iota_b, op=Alu.is_equal)

        # word/npi gathered rows, fp8, [P, {word,npi}, COLS, H]
        wn = singles.tile([P, 2, COLS, H], fp8)
        NG = S // 4  # 512 idxs per gather chunk
        for quarter in range(4):
            for gi, (tbl, idxs) in enumerate(((d_word, wi16), (d_npi, ni16))):
                nc.gpsimd.dma_gather(
                    out_ap=wn[:, gi, quarter * 4:(quarter + 1) * 4, :],
                    in_ap=tbl.ap(), idxs_ap=idxs[:, quarter, :],
                    num_idxs=NG, num_idxs_reg=NG, elem_size=H,
                    queue_num=gi,
                )

        def affine_segs(r0, r1):
            w = r1 - r0
            mod_end = max(r0, min(r1, MODW)) - r0
            cuts = sorted({0, w} | {n for n in (512, 1024) if 0 < n < w} | ({mod_end} if 0 < mod_end < w else set()))
            return list(zip(cuts[:-1], cuts[1:])), mod_end

        # Software-pipelined main loop: for each column, the Time2Vec
        # affine/reduce/sin runs one column AHEAD of the embedding-sum +
        # LayerNorm, so the PE always has ready matmul work while DVE/ACT
        # process the previous stage.  All PSUM tiles rotate through one
        # 4-buffer tag (4 x 2 banks = all 8 PSUM banks).
        NCHUNKS = ((0, 512), (512, H))
        sin_of, ps_of = {}, {}

        def t2v_front(c):
            """affines (R3,R1,R2) + sin-R3 + k-passes for column c."""
            sinS = sins.tile([P, TW], fp8, tag="sinS")
            ps_r, kt_r, me_r, segs_r = {}, {}, {}, {}
            for ri in (2, 0, 1):
                r0, r1 = ROUNDS[ri]
                ps = pst.tile([P, 1024], f32, tag="ps")
                ps_r[ri] = ps
                segs, mod_end = affine_segs(r0, r1)
                me_r[ri], segs_r[ri] = mod_end, segs
                last_in_reg = {}
                for (a0, a1) in segs:
                    last_in_reg[a0 // 512] = a0
                seen = set()
                for (a0, a1) in segs:
                    reg = a0 // 512
                    nc.tensor.matmul(
                        out=ps[:, a0:a1], lhsT=lhsT[:, c, :],
                        rhs=rhsT[:, r0 + a0:r0 + a1],
                        start=reg not in seen, stop=last_in_reg[reg] == a0,
                    )
                    seen.add(reg)
                if ri == 2:
                    nc.scalar.activation(
                        out=sinS[:, r0:r1], in_=ps[:, 0:r1 - r0],
                        func=Act.Sin, scale=TWO_PI,
                    )
                else:
                    # k-pass immediately after each round's affine so the
                    # DVE starts while the PE streams the next round
                    kt = kpool.tile([P, 1024], fp16, tag="kS")
                    kt_r[ri] = kt
                    nc.vector.tensor_scalar(
                        out=kt[:, 0:mod_end], in0=ps[:, 0:mod_end],
                        scalar1=12582912.0, scalar2=12582912.0,
                        op0=Alu.add, op1=Alu.subtract,
                    )
            return sinS, (ps_r, kt_r, me_r, segs_r)

        def t2v_back(c):
            """-I subtract matmuls + sins for column c."""
            ps_r, kt_r, me_r, segs_r = ps_of[c]
            for ri in (0, 1):
                for (a0, a1) in segs_r[ri]:
                    if a0 >= me_r[ri]:
                        break
                    nc.tensor.matmul(
                        out=ps_r[ri][:, a0:a1], lhsT=negi[:],
                        rhs=kt_r[ri][:, a0:a1], start=False, stop=True,
                        skip_group_check=True,
                    )
            for ri in (0, 1):
                r0, r1 = ROUNDS[ri]
                nc.scalar.activation(
                    out=sin_of[c][:, r0:r1], in_=ps_r[ri][:, 0:r1 - r0],
                    func=Act.Sin, scale=TWO_PI,
                )

        def emb_ln(c):
            """7-way sum on TensorE + LayerNorm + store for column c."""
            sinS = sin_of.pop(c)
            psE = pst.tile([P, 1024], f32, tag="ps")
            for n0, n1 in NCHUNKS:
                nc.tensor.matmul(
                    out=psE[:, n0:n1], lhsT=onehot[:, c, :],
                    rhs=ctbl[:, n0:n1], start=True, stop=False,
                )
            wn_ap = wn[:]
            s_ap = sinS[:]
            for n0, n1 in NCHUNKS:
                n = n1 - n0
                nc.tensor.matmul(
                    out=psE[:, n0:n1],
                    lhsT=id2[:],
                    rhs=bass.AP(tensor=wn_ap.tensor,
                                offset=wn_ap.offset + c * H + n0,
                                ap=[wn_ap.ap[0], [COLS * H, 2], [1, n]]),
                    start=False, stop=False, perf_mode=DR,
                )
                na = min(n1, HA) - n0  # age/del sins are 767 wide
                nc.tensor.matmul(
                    out=psE[:, n0:n0 + na],
                    lhsT=id2[:],
                    rhs=bass.AP(tensor=s_ap.tensor, offset=s_ap.offset + n0,
                                ap=[s_ap.ap[0], [HA, 2], [1, na]]),
                    start=False, stop=False, perf_mode=DR,
                )
            for n0, n1 in NCHUNKS:
                nc.tensor.matmul(
                    out=psE[:, n0:n1], lhsT=idp[:],
                    rhs=sinS[:, 2 * HA + n0:2 * HA + n1],
                    start=False, stop=True,
                )
            nc.vector.tensor_tensor(
                out=psE[:, H - 1:H], in0=psE[:, H - 1:H],
                in1=v2t[:, c:c + 1], op=Alu.add,
            )
            # LN stats + rsqrt(var) via bit-trick + 1 Newton step
            st = small.tile([P, 2, 6], f32, tag="bnst")
            nc.vector.bn_stats(out=st[:, 0, :], in_=psE[:, 0:H // 2])
            nc.vector.bn_stats(out=st[:, 1, :], in_=psE[:, H // 2:H])
            mv = small.tile([P, 2], f32, tag="mv")
            nc.vector.bn_aggr(out=mv[:], in_=st[:].rearrange("p a b -> p (a b)"))
            vv = mv[:, 1:2]
            y = small.tile([P, 1], f32, tag="rsq_y")
            t = small.tile([P, 1], f32, tag="rsq_t")
            nc.vector.tensor_scalar(
                out=y[:].bitcast(i32), in0=vv.bitcast(i32),
                scalar1=1, scalar2=None, op0=Alu.logical_shift_right,
            )
            nc.vector.tensor_tensor(
                out=y[:].bitcast(i32), in0=magic[:, 0:1], in1=y[:].bitcast(i32),
                op=Alu.subtract,
            )
            # two fused Newton steps: t = (v*y)*y ; t = 1.5 - 0.5*t ; y *= t
            for _ in range(2):
                nc.vector.scalar_tensor_tensor(
                    out=t[:], in0=vv, scalar=y[:], in1=y[:],
                    op0=Alu.mult, op1=Alu.mult,
                )
                nc.vector.tensor_scalar(
                    out=t[:], in0=t[:], scalar1=-0.5, scalar2=1.5,
                    op0=Alu.mult, op1=Alu.add,
                )
                nc.vector.tensor_tensor(out=y[:], in0=y[:], in1=t[:], op=Alu.mult)
            nmu = small.tile([P, 1], f32, tag="nmu")
            nc.vector.scalar_tensor_tensor(
                out=nmu[:], in0=mv[:, 0:1], scalar=-1.0, in1=y[:],
                op0=Alu.mult, op1=Alu.mult,
            )
            outS = opool.tile([P, H], f32, tag="outS")
            nc.scalar.activation(
                out=outS[:], in_=psE[:, 0:H], func=Act.Identity,
                scale=y[:], bias=nmu[:],
            )
            if use_gamma_beta:
                nc.vector.tensor_tensor(out=outS[:], in0=outS[:], in1=gamma_t[:], op=Alu.mult)
                nc.vector.tensor_tensor(out=outS[:], in0=outS[:], in1=beta_t[:], op=Alu.add)
            nc.sync.dma_start(
                out=d_out.ap().rearrange("(c p) h -> p c h", p=P)[:, c, :],
                in_=outS[:],
            )

        for c in range(COLS + 1):
            if c < COLS:
                sin_of[c], ps_of[c] = t2v_front(c)
            if c >= 1:
                emb_ln(c - 1)
            if c < COLS:
                t2v_back(c)
                ps_of.pop(c, None)

    nc.compile()
    return nc


def _get_nc(use_gamma_beta: bool):
    key = ("nc", use_gamma_beta)
    if key not in _cache:
        _cache[key] = _build(use_gamma_beta)
    return _cache[key]


def _f32a(x):
    return np.ascontiguousarray(np.asarray(x), dtype=np.float32)


def _i32a(x):
    return np.ascontiguousarray(np.asarray(x), dtype=np.int32)


def _pack_idx16(ids_row):
    # [S] -> [P, 4, 32]: idx position i of quarter q at [i % 16, q, i // 16], x8.
    QN = S // 4
    arr = np.zeros((16, 4, QN // 16), dtype=np.int16)
    for q in range(4):
        blk = ids_row[q * QN:(q + 1) * QN].reshape(QN // 16, 16)
        arr[:, q, :] = blk.T.astype(np.int16)
    return np.ascontiguousarray(np.tile(arr, (8, 1, 1)))


def _split3(x, bf):
    x = np.asarray(x, dtype=np.float64)
    p1 = x.astype(bf)
    r = x - p1.astype(np.float64)
    p2 = r.astype(bf)
    p3 = (r - p2.astype(np.float64)).astype(bf)
    return p1, p2, p3


_last_use_gb = False


def _make_in_maps(inputs, use_gb):
    import ml_dtypes
    bf = ml_dtypes.bfloat16
    f8 = ml_dtypes.float8_e4m3

    word_ids = _i32a(inputs["word_ids"]).reshape(B, S)
    modal_ids = _i32a(inputs["modalities_ids"]).reshape(B, S)
    seg_ids = _i32a(inputs["seg_ids"]).reshape(B, S)
    npi_ids = _i32a(inputs["npi_ids"]).reshape(B, S)
    posi_ids = _i32a(inputs["posi_ids"]).reshape(B, S)
    age_tau = _f32a(inputs["age_tau"]).reshape(B, S)
    delay_tau = _f32a(inputs["delays_tau"]).reshape(B, S)

    # ---- rhs20: split-precision weight rows (shared across cores) ----
    aw = _f32a(inputs["age_w"]).reshape(HA) / TWO_PI
    ab = _f32a(inputs["age_b"]).reshape(HA) / TWO_PI
    dw = _f32a(inputs["delay_w"]).reshape(HA) / TWO_PI
    db = _f32a(inputs["delay_b"]).reshape(HA) / TWO_PI
    j = np.arange(H, dtype=np.float64)
    omega = (10000.0 ** (-2.0 * j / H)) / TWO_PI      # turns per unit pos
    sign = np.where(j % 2 == 0, 1.0, -1.0)            # odd dims: cos via 0.25 - x
    phase = np.where(j % 2 == 0, 0.0, 0.25)
    som = sign * omega
    # slot order: lo dims (j < JLO) then hi dims
    order = np.concatenate([j[:JLO], j[JLO:]]).astype(np.int64)
    som_s, phase_s = som[order], phase[order]

    aw1, aw2, aw3 = _split3(aw, bf)
    dw1, dw2, dw3 = _split3(dw, bf)
    ab1, ab2, ab3 = _split3(ab, bf)
    db1, db2, db3 = _split3(db, bf)
    om1, om2, om3 = _split3(som_s, bf)

    rhs = np.zeros((K20, TW), dtype=bf)
    rhs[0, 0:HA], rhs[1, 0:HA], rhs[2, 0:HA] = aw1, aw2, aw3
    rhs[3, 0:HA], rhs[4, 0:HA] = aw1, aw2
    rhs[5, 0:HA] = aw1
    rhs[6, HA:2 * HA], rhs[7, HA:2 * HA], rhs[8, HA:2 * HA] = dw1, dw2, dw3
    rhs[9, HA:2 * HA], rhs[10, HA:2 * HA] = dw1, dw2
    rhs[11, HA:2 * HA] = dw1
    rhs[12, 2 * HA:], rhs[13, 2 * HA:], rhs[14, 2 * HA:] = om1, om2, om3
    rhs[15, 2 * HA:], rhs[16, 2 * HA:] = om1, om2
    rhs[17, 0:HA], rhs[18, 0:HA], rhs[19, 0:HA] = ab1, ab2, ab3
    rhs[17, HA:2 * HA], rhs[18, HA:2 * HA], rhs[19, HA:2 * HA] = db1, db2, db3
    rhs[17, 2 * HA:] = phase_s.astype(bf)

    # combined modal+seg table
    ctbl = np.zeros((KMS, H), dtype=bf)
    ctbl[0:MODAL_V] = _f32a(inputs["modalities_table"]).reshape(MODAL_V, H).astype(bf)
    ctbl[32:32 + SEG_V] = _f32a(inputs["seg_table"]).reshape(SEG_V, H).astype(bf)
    iota64 = np.full((KMS, 1), -1, dtype=np.int32)
    iota64[0:MODAL_V, 0] = np.arange(MODAL_V)
    iota64[32:32 + SEG_V, 0] = np.arange(SEG_V)

    id2 = np.zeros((P, 2, P), dtype=f8)
    eye = np.eye(P, dtype=np.float32)
    id2[:, 0, :] = eye.astype(f8)
    id2[:, 1, :] = eye.astype(f8)
    identp = np.ascontiguousarray(eye.astype(f8))
    negi = np.ascontiguousarray((-eye).astype(np.float16))

    shared = {
        "rhs20": np.ascontiguousarray(rhs),
        "word_table": np.ascontiguousarray(
            _f32a(inputs["word_table"]).reshape(VOCAB, H).astype(f8)),
        "npi_table": np.ascontiguousarray(
            _f32a(inputs["npi_table"]).reshape(NPI_V, H).astype(f8)),
        "ctbl": np.ascontiguousarray(ctbl),
        "iota64": iota64,
        "ident2": np.ascontiguousarray(id2),
        "identp": identp,
        "negident": negi,
    }
    if use_gb:
        shared["ln_gamma"] = _f32a(inputs["ln_gamma"]).reshape(H)
        shared["ln_beta"] = _f32a(inputs["ln_beta"]).reshape(H)

    aw0 = float(_f32a(inputs["age_w0"]).reshape(()))
    ab0 = float(_f32a(inputs["age_b0"]).reshape(()))
    dw0 = float(_f32a(inputs["delay_w0"]).reshape(()))
    db0 = float(_f32a(inputs["delay_b0"]).reshape(()))

    in_maps = []
    for i in range(B):
        ta = age_tau[i].astype(np.float64)
        td = delay_tau[i].astype(np.float64)
        pos = posi_ids[i].astype(np.float64)
        t1a, t2a, t3a = _split3(ta, bf)
        t1d, t2d, t3d = _split3(td, bf)
        p1 = pos.astype(bf)
        p2 = (pos - p1.astype(np.float64)).astype(bf)
        ones = np.ones(S, dtype=bf)
        lhsT = np.stack([
            t1a, t1a, t1a, t2a, t2a, t3a,
            t1d, t1d, t1d, t2d, t2d, t3d,
            p1, p1, p1, p2, p2,
            ones, ones, ones,
        ]).astype(bf)
        v2 = (ta * aw0 + ab0 + td * dw0 + db0).astype(np.float32)
        v2t = np.ascontiguousarray(v2.reshape(COLS, P).T)
        m = dict(shared)
        m.update(
            lhsT20=np.ascontiguousarray(lhsT),
            word_idx16=_pack_idx16(word_ids[i]),
            npi_idx16=_pack_idx16(npi_ids[i]),
            modalities_ids=modal_ids[i],
            seg_ids=seg_ids[i],
            v2t=v2t,
        )
        in_maps.append(m)
    return in_maps


def kernel(**inputs) -> np.ndarray:
    global _last_use_gb
    from concourse.bass_utils import run_bass_kernel_spmd

    gamma = _f32a(inputs["ln_gamma"])
    beta = _f32a(inputs["ln_beta"])
    use_gb = not (np.all(gamma == 1.0) and np.all(beta == 0.0))
    _last_use_gb = use_gb
    nc = _get_nc(use_gb)
    in_maps = _make_in_maps(inputs, use_gb)
    core_ids = list(range(B))
    res = run_bass_kernel_spmd(nc, in_maps, core_ids)
    out = np.stack([res.results[i]["out"] for i in core_ids], axis=0)
    return out


# revision 31
# speedup vs baseline: 1.3532x; 1.0306x over previous
"""BertEmbeddings (7-way embedding sum + Time2Vec + LayerNorm) on 8 TRN2 cores.

Redesign v3: all Time2Vec / sinusoidal-position math is computed on-chip
with the heavy lifting moved off the Vector engine:

  - The affine angle x = tau*w' + b' (in TURNS) is computed on the Tensor
    engine as a K=20 bf16 split-precision matmul (each operand split into
    2-3 bf16 parts whose pairwise products are exact) -> psT (PSUM f32).
  - Range reduction: ONE DVE tensor_scalar pass (x + 2^23) - 2^23 = round(x)
    (f32 RNE mantissa trick) -> k (fp16, exact for |k|<2048); then a second
    matmul with lhsT = -I accumulates -k INTO psT, leaving d = x - round(x)
    in [-0.5, 0.5].  No mod/cast/subtract passes on DVE.
  - ACT Sin reads PSUM directly: sin(2*pi*d), |arg| <= pi (LUT-valid).
    Position features with small angles (dims >= 271) skip reduction
    entirely (phases arranged so args always land in [-pi, pi]).
  - Word/NPI rows are dma_gather'ed as fp8(e4m3); sin outputs are fp8.
    The 7-way sum runs on TensorE: one K=64 one-hot matmul (modal+seg),
    one DoubleRow fp8 matmul summing word+npi in a single pass, one
    DoubleRow matmul summing age+del sins, one plain fp8 matmul for posi.
  - LayerNorm: bn_stats/bn_aggr (DVE), rstd via Quake-style bit-trick +
    2 Newton steps (no Sqrt ACT table -> the Sin table set stays loaded),
    normalize on ACT (scale/bias per partition), DMA out per column.

Self-contained: hardcodes shapes; kernel(**inputs) takes full unsharded
inputs, returns the full [8, 2048, 768] float32 output.
"""

import math

import numpy as np

B, S, H = 8, 2048, 768
VOCAB, MODAL_V, SEG_V, NPI_V, MAX_POS = 32000, 16, 4, 10000, 2048
P = 128
COLS = S // P            # 16 token-columns of 128 tokens
LN_EPS = 1e-12
TWO_PI = 2.0 * math.pi
KMS = 64                 # modal rows 0..15, seg rows 32..35, pad
K20 = 20                 # t2v matmul contraction rows
HA = H - 1               # 767 sin dims per t2v table
JLO = 271                # posi dims < JLO need range reduction
NLO = JLO                # width of posi_lo block
TW = HA + HA + H         # 2302 total t2v/posi slots: [age|del|posi_lo|posi_hi]
MODW = HA + HA + NLO     # 1805 slots that need range reduction
ROUNDS = [(0, 1024), (1024, 2048), (2048, TW)]
MAGIC = 0x5F3759DF
DEBUG_DUMP = False
LDW_OPT = False  # walrus ldw-opt crashes codegen (visitInstLdweights); keep off

_cache = {}


def _enable_ldw_opt():
    import concourse.bass_utils as bu

    if getattr(bu, "_ldw_patched", False):
        return
    orig = bu.run_command

    def run_command_ldw(cmd, *a, **kw):
        if isinstance(cmd, list):
            cmd = [("--enable-ldw-opt=true" if c == "--enable-ldw-opt=false" else c)
                   for c in cmd]
        return orig(cmd, *a, **kw)

    bu.run_command = run_command_ldw
    bu._ldw_patched = True


def _resplit_last(ap_obj, groups, width):
    """[P, N] AP -> [P, groups, width] AP (N = groups*width, contiguous)."""
    import concourse.bass as bass

    a = ap_obj
    assert a.ap[-1][0] == 1
    return bass.AP(
        tensor=a.tensor, offset=a.offset,
        ap=[a.ap[0], [width, groups], [1, width]],
    )


def _build(use_gamma_beta: bool):
    if LDW_OPT:
        _enable_ldw_opt()
    import concourse.bacc as bacc
    import concourse.bass as bass
    import concourse.tile as tile
    from concourse import mybir
    from contextlib import ExitStack

    f32 = mybir.dt.float32
    bf16 = mybir.dt.bfloat16
    fp16 = mybir.dt.float16
    fp8 = mybir.dt.float8e4
    i32 = mybir.dt.int32
    i16 = mybir.dt.int16
    Alu = mybir.AluOpType
    Act = mybir.ActivationFunctionType
    DR = mybir.MatmulPerfMode.DoubleRow

    nc = bacc.Bacc("TRN2", target_bir_lowering=False, debug=False,
                   dynamic_dma_scratch_size=24576, num_swdge_queues=2)

    d_lhsT = nc.dram_tensor("lhsT20", [K20, S], bf16, kind="ExternalInput")
    d_rhs = nc.dram_tensor("rhs20", [K20, TW], bf16, kind="ExternalInput")
    d_widx = nc.dram_tensor("word_idx16", [P, 4, S // 4 // 16], i16, kind="ExternalInput")
    d_nidx = nc.dram_tensor("npi_idx16", [P, 4, S // 4 // 16], i16, kind="ExternalInput")
    d_word = nc.dram_tensor("word_table", [VOCAB, H], fp8, kind="ExternalInput")
    d_npi = nc.dram_tensor("npi_table", [NPI_V, H], fp8, kind="ExternalInput")
    d_ctbl = nc.dram_tensor("ctbl", [KMS, H], bf16, kind="ExternalInput")
    d_modal_ids = nc.dram_tensor("modalities_ids", [S], i32, kind="ExternalInput")
    d_seg_ids = nc.dram_tensor("seg_ids", [S], i32, kind="ExternalInput")
    d_iota = nc.dram_tensor("iota64", [KMS, 1], i32, kind="ExternalInput")
    d_v2t = nc.dram_tensor("v2t", [P, COLS], f32, kind="ExternalInput")
    d_id2 = nc.dram_tensor("ident2", [P, 2, P], fp8, kind="ExternalInput")
    d_idp = nc.dram_tensor("identp", [P, P], fp8, kind="ExternalInput")
    d_negi = nc.dram_tensor("negident", [P, P], fp16, kind="ExternalInput")
    if use_gamma_beta:
        d_gamma = nc.dram_tensor("ln_gamma", [H], f32, kind="ExternalInput")
        d_beta = nc.dram_tensor("ln_beta", [H], f32, kind="ExternalInput")
    d_out = nc.dram_tensor("out", [S, H], f32, kind="ExternalOutput")
    if DEBUG_DUMP:
        d_dbg_sin = nc.dram_tensor("dbg_sin", [P, TW], f32, kind="ExternalOutput")
        d_dbg_emb = nc.dram_tensor("dbg_emb", [P, H], f32, kind="ExternalOutput")
        d_dbg_mv = nc.dram_tensor("dbg_mv", [P, 2, 2], f32, kind="ExternalOutput")
        d_dbg_wn = nc.dram_tensor("dbg_wn", [P, 2, H], f32, kind="ExternalOutput")
        d_dbg_oh = nc.dram_tensor("dbg_oh", [KMS, P], f32, kind="ExternalOutput")
        d_dbg_k = nc.dram_tensor("dbg_k", [P, 1024], f32, kind="ExternalOutput")
        d_dbg_d = nc.dram_tensor("dbg_d", [P, 1024], f32, kind="ExternalOutput")

    def bcast_rows(handle, n, count, offset=0):
        ap = handle.ap()
        return bass.AP(tensor=ap.tensor, offset=offset, ap=[[0, n], [1, count]])

    with tile.TileContext(nc) as tc, ExitStack() as ctx:
        singles = ctx.enter_context(tc.tile_pool(name="singles", bufs=1))
        sins = ctx.enter_context(tc.tile_pool(name="sins", bufs=16))
        kpool = ctx.enter_context(tc.tile_pool(name="kpool", bufs=3))
        opool = ctx.enter_context(tc.tile_pool(name="opool", bufs=6))
        small = ctx.enter_context(tc.tile_pool(name="small", bufs=2))
        pst = ctx.enter_context(tc.tile_pool(name="pst", bufs=4, space="PSUM"))

        # ---- static tiles ----
        lhsT = singles.tile([K20, COLS, P], bf16)
        nc.sync.dma_start(out=lhsT[:], in_=d_lhsT.ap().rearrange("k (c p) -> k c p", p=P))
        rhsT = singles.tile([K20, TW], bf16)
        nc.sync.dma_start(out=rhsT[:], in_=d_rhs.ap())
        id2 = singles.tile([P, 2, P], fp8)
        nc.sync.dma_start(out=id2[:], in_=d_id2.ap())
        idp = singles.tile([P, P], fp8)
        nc.sync.dma_start(out=idp[:], in_=d_idp.ap())
        negi = singles.tile([P, P], fp16)
        nc.sync.dma_start(out=negi[:], in_=d_negi.ap())
        ctbl = singles.tile([KMS, H], bf16)
        nc.sync.dma_start(out=ctbl[:], in_=d_ctbl.ap())
        v2t = singles.tile([P, COLS], f32)
        nc.sync.dma_start(out=v2t[:], in_=d_v2t.ap())
        iota = singles.tile([KMS, 1], i32)
        nc.sync.dma_start(out=iota[:], in_=d_iota.ap())
        magic = singles.tile([P, 2], i32)
        nc.vector.memset(magic[:], MAGIC)
        if use_gamma_beta:
            gamma_t = singles.tile([P, H], f32)
            beta_t = singles.tile([P, H], f32)
            nc.sync.dma_start(out=gamma_t[:], in_=bcast_rows(d_gamma, P, H))
            nc.sync.dma_start(out=beta_t[:], in_=bcast_rows(d_beta, P, H))

        # gather indices
        wi16 = singles.tile([P, 4, S // 4 // 16], i16)
        ni16 = singles.tile([P, 4, S // 4 // 16], i16)
        nc.sync.dma_start(out=wi16[:], in_=d_widx.ap())
        nc.sync.dma_start(out=ni16[:], in_=d_nidx.ap())

        # modal/seg ids broadcast to the one-hot orientation, one-hot built once
        ids_all = singles.tile([KMS, COLS, P], i32)
        nc.sync.dma_start(
            out=ids_all[0:MODAL_V, :, :],
            in_=bass.AP(tensor=d_modal_ids.ap().tensor, offset=0,
                        ap=[[0, MODAL_V], [P, COLS], [1, P]]),
        )
        nc.sync.dma_start(
            out=ids_all[32:32 + SEG_V, :, :],
            in_=bass.AP(tensor=d_seg_ids.ap().tensor, offset=0,
                        ap=[[0, SEG_V], [P, COLS], [1, P]]),
        )
        # rows not in {0..15, 32..35} compare against -1 -> all zeros
        nc.sync.dma_start(
            out=ids_all[16:32, :, :],
            in_=bass.AP(tensor=d_modal_ids.ap().tensor, offset=0,
                        ap=[[0, 16], [P, COLS], [1, P]]),
        )
        nc.sync.dma_start(
            out=ids_all[36:KMS, :, :],
            in_=bass.AP(tensor=d_modal_ids.ap().tensor, offset=0,
                        ap=[[0, KMS - 36], [P, COLS], [1, P]]),
        )
        onehot = singles.tile([KMS, COLS, P], bf16)
        iota_b = bass.AP(tensor=iota[:].tensor, offset=iota[:].offset,
                         ap=[iota[:].ap[0], [0, COLS], [0, P]])
        nc.vector.tensor_tensor(out=onehot[:], in0=ids_all[:], in1=iota_b, op=Alu.is_equal)

        # word/npi gathered rows, fp8, [P, {word,npi}, COLS, H]
        wn = singles.tile([P, 2, COLS, H], fp8)
        NG = S // 4  # 512 idxs per gather chunk
        for quarter in range(4):
            for gi, (tbl, idxs) in enumerate(((d_word, wi16), (d_npi, ni16))):
                nc.gpsimd.dma_gather(
                    out_ap=wn[:, gi, quarter * 4:(quarter + 1) * 4, :],
                    in_ap=tbl.ap(), idxs_ap=idxs[:, quarter, :],
                    num_idxs=NG, num_idxs_reg=NG, elem_size=H,
                    queue_num=gi,
                )

        def affine_segs(r0, r1):
            w = r1 - r0
            mod_end = max(r0, min(r1, MODW)) - r0
            cuts = sorted({0, w} | {n for n in (512, 1024) if 0 < n < w} | ({mod_end} if 0 < mod_end < w else set()))
            return list(zip(cuts[:-1], cuts[1:])), mod_end

        # Software-pipelined main loop: for each column, the Time2Vec
        # affine/reduce/sin runs one column AHEAD of the embedding-sum +
        # LayerNorm, so the PE always has ready matmul work while DVE/ACT
        # process the previous stage.  All PSUM tiles rotate through one
        # 4-buffer tag (4 x 2 banks = all 8 PSUM banks).
        NCHUNKS = ((0, 512), (512, H))
        sin_of, ps_of = {}, {}

        def t2v_front(c):
            """affines (R3,R1,R2) + sin-R3 + k-passes for column c."""
            sinS = sins.tile([P, TW], fp8, tag="sinS")
            ps_r, kt_r, me_r, segs_r = {}, {}, {}, {}
            for ri in (2, 0, 1):
                r0, r1 = ROUNDS[ri]
                ps = pst.tile([P, 1024], f32, tag="ps")
                ps_r[ri] = ps
                segs, mod_end = affine_segs(r0, r1)
                me_r[ri], segs_r[ri] = mod_end, segs
                last_in_reg = {}
                for (a0, a1) in segs:
                    last_in_reg[a0 // 512] = a0
                seen = set()
                for (a0, a1) in segs:
                    reg = a0 // 512
                    nc.tensor.matmul(
                        out=ps[:, a0:a1], lhsT=lhsT[:, c, :],
                        rhs=rhsT[:, r0 + a0:r0 + a1],
                        start=reg not in seen, stop=last_in_reg[reg] == a0,
                    )
                    seen.add(reg)
                if ri == 2:
                    nc.scalar.activation(
                        out=sinS[:, r0:r1], in_=ps[:, 0:r1 - r0],
                        func=Act.Sin, scale=TWO_PI,
                    )
                else:
                    # k-pass immediately after each round's affine so the
                    # DVE starts while the PE streams the next round
                    kt = kpool.tile([P, 1024], fp16, tag="kS")
                    kt_r[ri] = kt
                    nc.vector.tensor_scalar(
                        out=kt[:, 0:mod_end], in0=ps[:, 0:mod_end],
                        scalar1=12582912.0, scalar2=12582912.0,
                        op0=Alu.add, op1=Alu.subtract,
                    )
            return sinS, (ps_r, kt_r, me_r, segs_r)

        def t2v_back(c):
            """-I subtract matmuls + sins for column c."""
            ps_r, kt_r, me_r, segs_r = ps_of[c]
            for ri in (0, 1):
                for (a0, a1) in segs_r[ri]:
                    if a0 >= me_r[ri]:
                        break
                    nc.tensor.matmul(
                        out=ps_r[ri][:, a0:a1], lhsT=negi[:],
                        rhs=kt_r[ri][:, a0:a1], start=False, stop=True,
                        skip_group_check=True,
                    )
            for ri in (0, 1):
                r0, r1 = ROUNDS[ri]
                nc.scalar.activation(
                    out=sin_of[c][:, r0:r1], in_=ps_r[ri][:, 0:r1 - r0],
                    func=Act.Sin, scale=TWO_PI,
                )

        def emb_ln(c):
            """7-way sum on TensorE + LayerNorm + store for column c."""
            sinS = sin_of.pop(c)
            psE = pst.tile([P, 1024], f32, tag="ps")
            for n0, n1 in NCHUNKS:
                nc.tensor.matmul(
                    out=psE[:, n0:n1], lhsT=onehot[:, c, :],
                    rhs=ctbl[:, n0:n1], start=True, stop=False,
                )
            wn_ap = wn[:]
            s_ap = sinS[:]
            for n0, n1 in NCHUNKS:
                n = n1 - n0
                nc.tensor.matmul(
                    out=psE[:, n0:n1],
                    lhsT=id2[:],
                    rhs=bass.AP(tensor=wn_ap.tensor,
                                offset=wn_ap.offset + c * H + n0,
                                ap=[wn_ap.ap[0], [COLS * H, 2], [1, n]]),
                    start=False, stop=False, perf_mode=DR,
                )
                na = min(n1, HA) - n0  # age/del sins are 767 wide
                nc.tensor.matmul(
                    out=psE[:, n0:n0 + na],
                    lhsT=id2[:],
                    rhs=bass.AP(tensor=s_ap.tensor, offset=s_ap.offset + n0,
                                ap=[s_ap.ap[0], [HA, 2], [1, na]]),
                    start=False, stop=False, perf_mode=DR,
                )
            for n0, n1 in NCHUNKS:
                nc.tensor.matmul(
                    out=psE[:, n0:n1], lhsT=idp[:],
                    rhs=sinS[:, 2 * HA + n0:2 * HA + n1],
                    start=False, stop=True,
                )
            nc.vector.tensor_tensor(
                out=psE[:, H - 1:H], in0=psE[:, H - 1:H],
                in1=v2t[:, c:c + 1], op=Alu.add,
            )
            # LN stats + rsqrt(var) via bit-trick + 1 Newton step
            st = small.tile([P, 2, 6], f32, tag="bnst")
            nc.vector.bn_stats(out=st[:, 0, :], in_=psE[:, 0:H // 2])
            nc.vector.bn_stats(out=st[:, 1, :], in_=psE[:, H // 2:H])
            mv = small.tile([P, 2], f32, tag="mv")
            nc.vector.bn_aggr(out=mv[:], in_=st[:].rearrange("p a b -> p (a b)"))
            vv = mv[:, 1:2]
            y = small.tile([P, 1], f32, tag="rsq_y")
            t = small.tile([P, 1], f32, tag="rsq_t")
            nc.vector.tensor_scalar(
                out=y[:].bitcast(i32), in0=vv.bitcast(i32),
                scalar1=1, scalar2=None, op0=Alu.logical_shift_right,
            )
            nc.vector.tensor_tensor(
                out=y[:].bitcast(i32), in0=magic[:, 0:1], in1=y[:].bitcast(i32),
                op=Alu.subtract,
            )
            # two fused Newton steps: t = (v*y)*y ; t = 1.5 - 0.5*t ; y *= t
            for _ in range(2):
                nc.vector.scalar_tensor_tensor(
                    out=t[:], in0=vv, scalar=y[:], in1=y[:],
                    op0=Alu.mult, op1=Alu.mult,
                )
                nc.vector.tensor_scalar(
                    out=t[:], in0=t[:], scalar1=-0.5, scalar2=1.5,
                    op0=Alu.mult, op1=Alu.add,
                )
                nc.vector.tensor_tensor(out=y[:], in0=y[:], in1=t[:], op=Alu.mult)
            nmu = small.tile([P, 1], f32, tag="nmu")
            nc.vector.scalar_tensor_tensor(
                out=nmu[:], in0=mv[:, 0:1], scalar=-1.0, in1=y[:],
                op0=Alu.mult, op1=Alu.mult,
            )
            outS = opool.tile([P, H], f32, tag="outS")
            nc.scalar.activation(
                out=outS[:], in_=psE[:, 0:H], func=Act.Identity,
                scale=y[:], bias=nmu[:],
            )
            if use_gamma_beta:
                nc.vector.tensor_tensor(out=outS[:], in0=outS[:], in1=gamma_t[:], op=Alu.mult)
                nc.vector.tensor_tensor(out=outS[:], in0=outS[:], in1=beta_t[:], op=Alu.add)
            nc.sync.dma_start(
                out=d_out.ap().rearrange("(c p) h -> p c h", p=P)[:, c, :],
                in_=outS[:],
            )

        for c in range(COLS):
            sin_of[c], ps_of[c] = t2v_front(c)
            t2v_back(c)
            ps_of.pop(c, None)
        for c in range(COLS):
            emb_ln(c)

    nc.compile()
    return nc


def _get_nc(use_gamma_beta: bool):
    key = ("nc", use_gamma_beta)
    if key not in _cache:
        _cache[key] = _build(use_gamma_beta)
    return _cache[key]


def _f32a(x):
    return np.ascontiguousarray(np.asarray(x), dtype=np.float32)


def _i32a(x):
    return np.ascontiguousarray(np.asarray(x), dtype=np.int32)


def _pack_idx16(ids_row):
    # [S] -> [P, 4, 32]: idx position i of quarter q at [i % 16, q, i // 16], x8.
    QN = S // 4
    arr = np.zeros((16, 4, QN // 16), dtype=np.int16)
    for q in range(4):
        blk = ids_row[q * QN:(q + 1) * QN].reshape(QN // 16, 16)
        arr[:, q, :] = blk.T.astype(np.int16)
    return np.ascontiguousarray(np.tile(arr, (8, 1, 1)))


def _split3(x, bf):
    x = np.asarray(x, dtype=np.float64)
    p1 = x.astype(bf)
    r = x - p1.astype(np.float64)
    p2 = r.astype(bf)
    p3 = (r - p2.astype(np.float64)).astype(bf)
    return p1, p2, p3


_last_use_gb = False


def _make_in_maps(inputs, use_gb):
    import ml_dtypes
    bf = ml_dtypes.bfloat16
    f8 = ml_dtypes.float8_e4m3

    word_ids = _i32a(inputs["word_ids"]).reshape(B, S)
    modal_ids = _i32a(inputs["modalities_ids"]).reshape(B, S)
    seg_ids = _i32a(inputs["seg_ids"]).reshape(B, S)
    npi_ids = _i32a(inputs["npi_ids"]).reshape(B, S)
    posi_ids = _i32a(inputs["posi_ids"]).reshape(B, S)
    age_tau = _f32a(inputs["age_tau"]).reshape(B, S)
    delay_tau = _f32a(inputs["delays_tau"]).reshape(B, S)

    # ---- rhs20: split-precision weight rows (shared across cores) ----
    aw = _f32a(inputs["age_w"]).reshape(HA) / TWO_PI
    ab = _f32a(inputs["age_b"]).reshape(HA) / TWO_PI
    dw = _f32a(inputs["delay_w"]).reshape(HA) / TWO_PI
    db = _f32a(inputs["delay_b"]).reshape(HA) / TWO_PI
    j = np.arange(H, dtype=np.float64)
    omega = (10000.0 ** (-2.0 * j / H)) / TWO_PI      # turns per unit pos
    sign = np.where(j % 2 == 0, 1.0, -1.0)            # odd dims: cos via 0.25 - x
    phase = np.where(j % 2 == 0, 0.0, 0.25)
    som = sign * omega
    # slot order: lo dims (j < JLO) then hi dims
    order = np.concatenate([j[:JLO], j[JLO:]]).astype(np.int64)
    som_s, phase_s = som[order], phase[order]

    aw1, aw2, aw3 = _split3(aw, bf)
    dw1, dw2, dw3 = _split3(dw, bf)
    ab1, ab2, ab3 = _split3(ab, bf)
    db1, db2, db3 = _split3(db, bf)
    om1, om2, om3 = _split3(som_s, bf)

    rhs = np.zeros((K20, TW), dtype=bf)
    rhs[0, 0:HA], rhs[1, 0:HA], rhs[2, 0:HA] = aw1, aw2, aw3
    rhs[3, 0:HA], rhs[4, 0:HA] = aw1, aw2
    rhs[5, 0:HA] = aw1
    rhs[6, HA:2 * HA], rhs[7, HA:2 * HA], rhs[8, HA:2 * HA] = dw1, dw2, dw3
    rhs[9, HA:2 * HA], rhs[10, HA:2 * HA] = dw1, dw2
    rhs[11, HA:2 * HA] = dw1
    rhs[12, 2 * HA:], rhs[13, 2 * HA:], rhs[14, 2 * HA:] = om1, om2, om3
    rhs[15, 2 * HA:], rhs[16, 2 * HA:] = om1, om2
    rhs[17, 0:HA], rhs[18, 0:HA], rhs[19, 0:HA] = ab1, ab2, ab3
    rhs[17, HA:2 * HA], rhs[18, HA:2 * HA], rhs[19, HA:2 * HA] = db1, db2, db3
    rhs[17, 2 * HA:] = phase_s.astype(bf)

    # combined modal+seg table
    ctbl = np.zeros((KMS, H), dtype=bf)
    ctbl[0:MODAL_V] = _f32a(inputs["modalities_table"]).reshape(MODAL_V, H).astype(bf)
    ctbl[32:32 + SEG_V] = _f32a(inputs["seg_table"]).reshape(SEG_V, H).astype(bf)
    iota64 = np.full((KMS, 1), -1, dtype=np.int32)
    iota64[0:MODAL_V, 0] = np.arange(MODAL_V)
    iota64[32:32 + SEG_V, 0] = np.arange(SEG_V)

    id2 = np.zeros((P, 2, P), dtype=f8)
    eye = np.eye(P, dtype=np.float32)
    id2[:, 0, :] = eye.astype(f8)
    id2[:, 1, :] = eye.astype(f8)
    identp = np.ascontiguousarray(eye.astype(f8))
    negi = np.ascontiguousarray((-eye).astype(np.float16))

    shared = {
        "rhs20": np.ascontiguousarray(rhs),
        "word_table": np.ascontiguousarray(
            _f32a(inputs["word_table"]).reshape(VOCAB, H).astype(f8)),
        "npi_table": np.ascontiguousarray(
            _f32a(inputs["npi_table"]).reshape(NPI_V, H).astype(f8)),
        "ctbl": np.ascontiguousarray(ctbl),
        "iota64": iota64,
        "ident2": np.ascontiguousarray(id2),
        "identp": identp,
        "negident": negi,
    }
    if use_gb:
        shared["ln_gamma"] = _f32a(inputs["ln_gamma"]).reshape(H)
        shared["ln_beta"] = _f32a(inputs["ln_beta"]).reshape(H)

    aw0 = float(_f32a(inputs["age_w0"]).reshape(()))
    ab0 = float(_f32a(inputs["age_b0"]).reshape(()))
    dw0 = float(_f32a(inputs["delay_w0"]).reshape(()))
    db0 = float(_f32a(inputs["delay_b0"]).reshape(()))

    in_maps = []
    for i in range(B):
        ta = age_tau[i].astype(np.float64)
        td = delay_tau[i].astype(np.float64)
        pos = posi_ids[i].astype(np.float64)
        t1a, t2a, t3a = _split3(ta, bf)
        t1d, t2d, t3d = _split3(td, bf)
        p1 = pos.astype(bf)
        p2 = (pos - p1.astype(np.float64)).astype(bf)
        ones = np.ones(S, dtype=bf)
        lhsT = np.stack([
            t1a, t1a, t1a, t2a, t2a, t3a,
            t1d, t1d, t1d, t2d, t2d, t3d,
            p1, p1, p1, p2, p2,
            ones, ones, ones,
        ]).astype(bf)
        v2 = (ta * aw0 + ab0 + td * dw0 + db0).astype(np.float32)
        v2t = np.ascontiguousarray(v2.reshape(COLS, P).T)
        m = dict(shared)
        m.update(
            lhsT20=np.ascontiguousarray(lhsT),
            word_idx16=_pack_idx16(word_ids[i]),
            npi_idx16=_pack_idx16(npi_ids[i]),
            modalities_ids=modal_ids[i],
            seg_ids=seg_ids[i],
            v2t=v2t,
        )
        in_maps.append(m)
    return in_maps


def kernel(**inputs) -> np.ndarray:
    global _last_use_gb
    from concourse.bass_utils import run_bass_kernel_spmd

    gamma = _f32a(inputs["ln_gamma"])
    beta = _f32a(inputs["ln_beta"])
    use_gb = not (np.all(gamma == 1.0) and np.all(beta == 0.0))
    _last_use_gb = use_gb
    nc = _get_nc(use_gb)
    in_maps = _make_in_maps(inputs, use_gb)
    core_ids = list(range(B))
    res = run_bass_kernel_spmd(nc, in_maps, core_ids)
    out = np.stack([res.results[i]["out"] for i in core_ids], axis=0)
    return out
